# revision 3
# baseline (speedup 1.0000x reference)
"""Trainium2 Bass kernel for nn_BSplineActivation — ap_gather, pipelined.

Same algorithm as the ap_gather kernel (16-replicated layout, [1025, 4]
fp16 centered-coeff table with zero edge rows, one SBUF-local ap_gather
per rep, fp16 Horner), plus:
  * the rep loop runs on-device (tc.For_i), so the reps-delta metric
    isolates pure device time;
  * the loop body holds TWO independent rep pipelines (A/B tile sets),
    letting the Tile scheduler overlap pipeline A's GPSIMD gather with
    pipeline B's VectorE Horner;
  * the three dtype converts run on ScalarE, off the VectorE critical
    path.
"""
import sys

sys.path.insert(0, "/opt/trn_rl_repo")

import numpy as np

import concourse.bacc as bacc
import concourse.mybir as mybir
import concourse.tile as tile
from concourse.bass_utils import run_bass_kernel_spmd

NCORES = 8
NPC = 32768
NG = 8
GPTS = NPC // NG         # 4096
P = 128
FW = NPC // P            # 256
NUM_KNOTS = 1024
DEGREE = 3
NI = NUM_KNOTS - 1
NROWS = NI + 2           # 1025 rows; 0 and 1024 are zeros
NPTS = NCORES * NPC

f32 = mybir.dt.float32
f16 = mybir.dt.float16
i32 = mybir.dt.int32
i16 = mybir.dt.int16
AL = mybir.AluOpType
AF = mybir.ActivationFunctionType

_KNOTS32 = np.linspace(-np.pi, np.pi, NUM_KNOTS).astype(np.float32)
_H64 = (float(_KNOTS32[-1]) - float(_KNOTS32[0])) / float(NI)
_INV_H = float(np.float32(1.0 / _H64))
_CB = float(np.float32(-float(_KNOTS32[0]) / _H64))
_CBM = float(np.float32(_CB + 0.5))


def _bspline_basis_f64(x, knots, degree):
    t = knots.astype(np.float64)
    n = t.shape[0] - 1
    xe = x[:, None]
    B = ((t[:-1] <= xe) & (xe < t[1:])).astype(np.float64)
    for k in range(1, degree + 1):
        d1 = t[k:n] - t[: n - k]
        d2 = t[k + 1 : n + 1] - t[1 : n - k + 1]
        w1 = np.where(d1 > 0, (xe - t[: n - k]) / np.where(d1 > 0, d1, 1.0), 0.0)
        w2 = np.where(d2 > 0, (t[k + 1 : n + 1] - xe) / np.where(d2 > 0, d2, 1.0), 0.0)
        B = w1 * B[:, : n - k] + w2 * B[:, 1 : n - k + 1]
    return B


def _build_table(weights: np.ndarray) -> np.ndarray:
    """[1025, 4] fp16: row i+1 = centered cubic coeffs of interval i."""
    w64 = weights.astype(np.float64)
    fr = np.array([0.0625, 0.3125, 0.6875, 0.9375])
    t64 = _KNOTS32.astype(np.float64)
    lo, wid = t64[:-1], t64[1:] - t64[:-1]
    xs = lo[:, None] + wid[:, None] * fr[None, :]
    ys = _bspline_basis_f64(xs.ravel(), _KNOTS32, DEGREE) @ w64
    ys = ys.reshape(NI, 4)
    zs = (xs - float(_KNOTS32[0])) / _H64
    us = zs - np.arange(NI)[:, None] - 0.5
    V = np.stack([us**k for k in range(4)], axis=-1)
    a = np.linalg.solve(V, ys[:, :, None])[:, :, 0]
    tab = np.zeros((NROWS, 4), dtype=np.float16)
    tab[1 : NI + 1, :] = a.astype(np.float16)
    return tab


_NC_CACHE = {}


def _build_nc(pairs: int = 1):
    """Body = 2 independent rep pipelines; device loop runs `pairs` times."""
    nc = bacc.Bacc("TRN2", target_bir_lowering=False, debug=False, num_devices=NCORES)
    x_d = nc.dram_tensor("x", [NG, GPTS], f32, kind="ExternalInput")
    xw_d = nc.dram_tensor("xw", [P, FW], f32, kind="ExternalInput")
    tab_d = nc.dram_tensor("tab", [1, NROWS * 4], f16, kind="ExternalInput")
    y_d = nc.dram_tensor("y", [NG, GPTS], f32, kind="ExternalOutput")
    with tile.TileContext(nc) as tc:
        with tc.tile_pool(name="sbuf", bufs=1) as pool:
            x16 = pool.tile([P, GPTS], f32)
            xw = pool.tile([P, FW], f32)
            tab = pool.tile([P, NROWS * 4], f16)
            nc.sync.dma_start(tab[:], tab_d.ap()[:].to_broadcast((P, NROWS * 4)))
            nc.sync.dma_start(xw[:], xw_d.ap()[:])
            for q in range(NG):
                nc.sync.dma_start(
                    x16[16 * q : 16 * q + 16, :],
                    x_d.ap()[q : q + 1, :].to_broadcast((16, GPTS)),
                )
            tv = tab[:].rearrange("p (r c) -> p r c", c=4)

            def tileset(sfx):
                shapes = {"zw": ([P, FW], f32), "idxwi": ([P, FW], i32),
                          "idx16": ([P, FW], i16), "zm": ([P, GPTS], f32),
                          "idxi": ([P, GPTS], i32),
                          "u16": ([P, GPTS], f16), "gath": ([P, GPTS * 4], f16),
                          "acc16": ([P, GPTS], f16)}
                return {k: pool.tile(s, d, name=f"{k}_{sfx}")
                        for k, (s, d) in shapes.items()}

            def rep(t):
                gv = t["gath"][:].rearrange("p (j c) -> p j c", c=4)
                # wrapped side: int16 gather indices
                nc.vector.tensor_scalar(out=t["zw"][:], in0=xw[:], scalar1=_INV_H,
                                        scalar2=_CBM, op0=AL.mult, op1=AL.add)
                nc.vector.tensor_scalar(out=t["zw"][:], in0=t["zw"][:], scalar1=-0.4997,
                                        scalar2=1023.999, op0=AL.max, op1=AL.min)
                nc.scalar.copy(t["idxwi"][:], t["zw"][:])       # round to nearest
                nc.scalar.copy(t["idx16"][:], t["idxwi"][:])
                nc.gpsimd.ap_gather(
                    out_ap=gv[:, :, :], in_ap=tv[:], idxs_ap=t["idx16"][:],
                    channels=P, num_elems=NROWS, d=4, num_idxs=GPTS)
                # replicated side: u = zm - row
                nc.vector.tensor_scalar(out=t["zm"][:], in0=x16[:], scalar1=_INV_H,
                                        scalar2=_CBM, op0=AL.mult, op1=AL.add)
                nc.vector.tensor_scalar(out=t["zm"][:], in0=t["zm"][:], scalar1=-0.4997,
                                        scalar2=1023.999, op0=AL.max, op1=AL.min)
                nc.scalar.copy(t["idxi"][:], t["zm"][:])
                idxf = t["idxi"][:].bitcast(f32)
                nc.scalar.copy(idxf, t["idxi"][:])      # in-place i32 -> f32
                nc.vector.tensor_tensor(out=t["zm"][:], in0=t["zm"][:], in1=idxf,
                                        op=AL.subtract)
                nc.scalar.copy(t["u16"][:], t["zm"][:])
                # Horner in fp16
                nc.vector.tensor_tensor(out=t["acc16"][:], in0=gv[:, :, 3], in1=t["u16"][:], op=AL.mult)
                nc.vector.tensor_tensor(out=t["acc16"][:], in0=t["acc16"][:], in1=gv[:, :, 2], op=AL.add)
                nc.vector.tensor_tensor(out=t["acc16"][:], in0=t["acc16"][:], in1=t["u16"][:], op=AL.mult)
                nc.vector.tensor_tensor(out=t["acc16"][:], in0=t["acc16"][:], in1=gv[:, :, 1], op=AL.add)
                nc.vector.tensor_tensor(out=t["acc16"][:], in0=t["acc16"][:], in1=t["u16"][:], op=AL.mult)
                nc.vector.tensor_tensor(out=t["zm"][:], in0=t["acc16"][:], in1=gv[:, :, 0], op=AL.add)

            tA, tB = tileset("a"), tileset("b")
            with tc.For_i(0, pairs, 1):
                rep(tA)
                rep(tB)
            for q in range(NG):
                nc.sync.dma_start(y_d.ap()[q : q + 1, :], tB["zm"][16 * q : 16 * q + 1, :])
    nc.compile()
    return nc


def _in_maps(x, weights):
    tab = _build_table(np.asarray(weights))
    tabf = np.ascontiguousarray(tab.reshape(1, NROWS * 4))
    xs = np.asarray(x, dtype=np.float32).reshape(NCORES, NG, GPTS // 16, 16)
    xw = np.ascontiguousarray(xs.transpose(0, 1, 3, 2).reshape(NCORES, P, FW))
    xg = np.ascontiguousarray(xs.reshape(NCORES, NG, GPTS))
    return [{"x": xg[c], "xw": xw[c], "tab": tabf} for c in range(NCORES)]


def kernel(x: np.ndarray, weights: np.ndarray) -> np.ndarray:
    if "nc" not in _NC_CACHE:
        _NC_CACHE["nc"] = _build_nc()
    nc = _NC_CACHE["nc"]
    res = run_bass_kernel_spmd(nc, _in_maps(x, weights), core_ids=list(range(NCORES)))
    y = np.stack([res.results[c]["y"] for c in range(NCORES)], axis=0)
    return y.reshape(NPTS, 1).astype(np.float32)


def estimate_hw_ns(x=None, weights=None, reps_hi: int = 2002, timing_reps: int = 10) -> int:
    """Device ns per rep: the body (2 reps) loops on-device via For_i, so
    wall(reps_hi) - wall(2) isolates device execution of reps_hi - 2 reps."""
    import time as _time

    if x is None:
        rng = np.random.default_rng(0)
        x = rng.standard_normal((NPTS, 1)).astype(np.float32)
        weights = rng.standard_normal((1020,)).astype(np.float32)
    im = _in_maps(x, weights)
    ncs = {}
    for pairs in (1, reps_hi // 2):
        nc = _NC_CACHE.get(("nc", pairs))
        if nc is None:
            nc = _build_nc(pairs) if pairs > 1 else _NC_CACHE.get("nc") or _build_nc()
            _NC_CACHE[("nc", pairs)] = nc
        ncs[pairs] = nc
        run_bass_kernel_spmd(nc, im, core_ids=list(range(NCORES)))

    def one(nc):
        t0 = _time.perf_counter()
        run_bass_kernel_spmd(nc, im, core_ids=list(range(NCORES)))
        return _time.perf_counter() - t0

    tl, th = [], []
    for _ in range(timing_reps):          # interleaved: common-mode drift cancels
        tl.append(one(ncs[1]))
        th.append(one(ncs[reps_hi // 2]))
    return int((min(th) - min(tl)) / (reps_hi - 2) * 1e9)
